# revision 5
# baseline (speedup 1.0000x reference)
"""Multi-head attention on 8 Trainium2 NeuronCores.

Sharding: 2-way data parallel over batch x 4-way tensor parallel over heads
(4 heads/core). Per-core device kernel, for its (batch, head-group):

  phase A : q^T = (x_q @ Wq + bq)^T, k^T likewise (feature-major, fp16),
            pipelined by token half so phase B starts after only the
            q-half0/k-half0 projections; the half-1 projections ride
            behind the first unit's score stream.
  phase A2: v = x_v @ Wv + bv (token-major, bf16, ones column appended)
  phase B : per (q-half, head) unit: s^T = k q^T (transposed-score layout),
            e^T = exp(s^T/8) (bf16), masked in place (DVE/Pool split), then
            PV in the swapped orientation: acc[q, 0:65] += em^T.T @ [v|1]
            per 128-token q-chunk -- the moving operand is the 65-wide
            [v|1], so PV costs 65 PE columns per (strip, chunk) instead of
            1024 per strip, and the ones column gives softmax row-sums for
            free, aligned per-partition.  Normalize via DVE reciprocal +
            per-partition tensor_scalar mul, then PE-transpose head-pair
            x tiles back to feature-major for phase C.
  phase C : partial_out = x^T.T @ Wo_rows (row-parallel Wo), interleaved.

Host: shards/transposes inputs (fp16), sums the 4 group partials per batch,
adds bo. fp16 matmul inputs run the PE at full rate; PSUM accumulation is
fp32; probabilities are bf16.
"""
import numpy as np
import ml_dtypes

import concourse.bass as bass
import concourse.bacc as bacc
import concourse.tile as tile
from concourse import mybir
from concourse.bass_utils import run_bass_kernel_spmd

B, S, D, H = 2, 2048, 1024, 16
DK = 64                    # head dim
GROUPS = 4                 # head-group tensor parallel factor
HL = H // GROUPS           # heads per core
DH = HL * DK               # 256 local features
NCORES = 8
NK = D // 128              # 8 contraction chunks
NJ = S // 128              # 16 token strips
SC = 512                   # matmul moving-operand chunk
HS = S // 2                # 1024: q-half size in phase B
NQC = HS // 128            # 8 q-chunks per half
F32 = mybir.dt.float32
F16 = mybir.dt.float16
BF16 = mybir.dt.bfloat16
AF = mybir.ActivationFunctionType

# mask-mul strips offloaded from DVE to the (otherwise idle) GPSIMD engine
POOL_JS = (2, 5, 8, 11, 14)

_CACHE = {}


def _build():
    nc = bacc.Bacc("TRN2")
    xqT = nc.dram_tensor("xqT", (D, S), F16, kind="ExternalInput")
    xkT = nc.dram_tensor("xkT", (D, S), F16, kind="ExternalInput")
    xvT = nc.dram_tensor("xvT", (D, S), F16, kind="ExternalInput")
    mT = nc.dram_tensor("mT", (S, S), BF16, kind="ExternalInput")
    wq = nc.dram_tensor("wq", (D, DH), F16, kind="ExternalInput")
    wk = nc.dram_tensor("wk", (D, DH), F16, kind="ExternalInput")
    wv = nc.dram_tensor("wv", (D + 1, DH), F16, kind="ExternalInput")
    wo = nc.dram_tensor("wo", (DH, D), F16, kind="ExternalInput")
    bqk = nc.dram_tensor("bqk", (128, 4), F32, kind="ExternalInput")
    ident = nc.dram_tensor("ident", (128, 128), F16, kind="ExternalInput")
    out = nc.dram_tensor("out", (S, D), BF16, kind="ExternalOutput")

    with tile.TileContext(nc) as tc:
        with tc.tile_pool(name="sp", bufs=1) as sp:
            qT = sp.tile([128, 2, S], F16)
            kT = sp.tile([128, 2, S], F16)
            vta = sp.tile([128, NJ, HL, DK + 1], BF16, name="vta")
            mTs = sp.tile([128, NJ, S], BF16)
            woS = sp.tile([128, 2, D], F16)
            xfin = sp.tile([128, 2, S], F16)
            identS = sp.tile([128, 128], F16)
            ones = sp.tile([1, SC], F16)
            zlhs = sp.tile([1, 128], F16)
            nc.vector.memset(ones, 1.0)
            nc.vector.memset(zlhs, 0.0)
            nc.vector.memset(vta[:, :, :, DK:DK + 1], 1.0)

            # early mask (q-half 0, first strips) rides the idle Pool queue
            for j in range(4):
                nc.gpsimd.dma_start(out=mTs[:, j, 0:HS],
                                    in_=mT[j * 128:(j + 1) * 128, 0:HS])

            biasT = sp.tile([128, 4], F32)
            nc.sync.dma_start(out=biasT, in_=bqk[:, :])
            nc.sync.dma_start(out=identS, in_=ident[:, :])

            pa2 = tc.alloc_tile_pool(name="pa2", bufs=1)
            wvS = pa2.tile([128, NK + 1, DH], F16, name="wv", bufs=1)

            pa = tc.alloc_tile_pool(name="pa", bufs=1)
            psA = tc.alloc_tile_pool(name="psA", bufs=1, space="PSUM")

            def load_xhalf(xT, half):
                """Stream one token-half of an input into 8 xc tiles."""
                off = half * HS
                xcs = []
                for kc in range(NK):
                    xc = pa.tile([128, HS], F16, name="xc", tag="xs", bufs=8)
                    nc.sync.dma_start(
                        out=xc, in_=xT[kc * 128:(kc + 1) * 128,
                                       off:off + HS])
                    xcs.append(xc)
                return xcs

            def proj_half(name, xcs, wS, dst, half, engs):
                """Project one token-half of q or k: psum [128, HS] per
                m-chunk (serial, projps bufs=1), evict with bias."""
                off = half * HS
                bc0 = 0 if name == "q" else 2
                for m in range(2):
                    ps = psA.tile([128, HS], F32, name=f"ps{name}{half}{m}",
                                  tag="projps", bufs=1)
                    for kc in range(NK):
                        for n in range(HS // SC):
                            nc.tensor.matmul(
                                out=ps[:, n * SC:(n + 1) * SC],
                                lhsT=wS[0:128, kc, m * 128:(m + 1) * 128],
                                rhs=xcs[kc][0:128, n * SC:(n + 1) * SC],
                                start=(kc == 0), stop=(kc == NK - 1))
                    if engs[m] == "act":
                        nc.scalar.activation(
                            dst[:, m, off:off + HS], ps, AF.Identity,
                            bias=biasT[:, bc0 + m:bc0 + m + 1])
                    else:
                        with nc.allow_low_precision(
                                reason="bias add into fp16 eviction"):
                            nc.vector.tensor_scalar_add(
                                dst[:, m, off:off + HS], ps,
                                biasT[:, bc0 + m:bc0 + m + 1])

            # ---- phase A, half 0 (critical path to first score) ----
            wqS = pa.tile([128, NK, DH], F16, name="wqS", tag="wq", bufs=1)
            for kc in range(NK):
                nc.sync.dma_start(out=wqS[0:128, kc, :],
                                  in_=wq[kc * 128:(kc + 1) * 128, :])
            xq0 = load_xhalf(xqT, 0)
            proj_half("q", xq0, wqS, qT, 0, ("act", "dve"))
            wkS = pa.tile([128, NK, DH], F16, name="wkS", tag="wk", bufs=1)
            for kc in range(NK):
                nc.sync.dma_start(out=wkS[0:128, kc, :],
                                  in_=wk[kc * 128:(kc + 1) * 128, :])
            xk0 = load_xhalf(xkT, 0)
            proj_half("k", xk0, wkS, kT, 0, ("act", "dve"))

            # need-ordered DMA for the rest (SP queue order = need order):
            # mask j4-9 (h0), xk-h1, mask j10-15 (h0), xq-h1, wv+xv, wo,
            # mask h1
            for j in range(4, 10):
                nc.sync.dma_start(out=mTs[:, j, 0:HS],
                                  in_=mT[j * 128:(j + 1) * 128, 0:HS])
            xk1 = load_xhalf(xkT, 1)
            for j in range(10, NJ):
                nc.sync.dma_start(out=mTs[:, j, 0:HS],
                                  in_=mT[j * 128:(j + 1) * 128, 0:HS])
            xq1 = load_xhalf(xqT, 1)
            for kc in range(NK + 1):
                p0 = kc * 128
                pc = min(128, D + 1 - p0)
                nc.sync.dma_start(out=wvS[0:pc, kc, :],
                                  in_=wv[p0:p0 + pc, :])
            xchs = [[None] * NK for _ in range(2)]
            for grp in range(2):
                goff = grp * HS
                for kc in range(NK):
                    xch = pa2.tile([128, HS], F16, name="xch",
                                   tag="xch", bufs=16)
                    nc.sync.dma_start(
                        out=xch,
                        in_=xvT[kc * 128:(kc + 1) * 128, goff:goff + HS])
                    xchs[grp][kc] = xch
            for s2 in range(2):
                nc.sync.dma_start(out=woS[:, s2, :],
                                  in_=wo[s2 * 128:(s2 + 1) * 128, :])
            for j in range(NJ):
                nc.sync.dma_start(out=mTs[:, j, HS:S],
                                  in_=mT[j * 128:(j + 1) * 128, HS:S])

            # ---------------- phase B ----------------
            psB = tc.alloc_tile_pool(name="psB", bufs=1, space="PSUM")
            pb = tc.alloc_tile_pool(name="pb", bufs=1)

            def emit_smem(h, half, j, po_, hs_):
                # scores -> exp -> in-place mask-mul for one (unit, j) strip
                off = half * HS
                sT = psB.tile([128, HS], F32, name="sT", tag="big", bufs=2)
                for c2 in range(2):
                    nc.tensor.matmul(
                        out=sT[:, c2 * SC:(c2 + 1) * SC],
                        lhsT=kT[po_:po_ + DK, hs_,
                                j * 128:(j + 1) * 128],
                        rhs=qT[po_:po_ + DK, hs_,
                               off + c2 * SC:off + (c2 + 1) * SC],
                        start=True, stop=True)
                eT = pb.tile([128, HS], BF16, name="eT", tag="eT", bufs=16)
                nc.scalar.activation(eT, sT, AF.Exp, scale=0.125)
                eng = nc.gpsimd if j in POOL_JS else nc.vector
                eng.tensor_mul(eT, eT, mTs[:, j, off:off + HS])
                return eT

            def emit_vstrip(m):
                # v projection for one 128-token strip (token-major + ones)
                grp, m8 = m // 8, m % 8
                pv = psB.tile([128, DH], F32, name="pv", tag="big", bufs=2)
                for kc in range(NK + 1):
                    if kc < NK:
                        lhsT = xchs[grp][kc][:, m8 * 128:(m8 + 1) * 128]
                    else:
                        lhsT = ones[0:1, 0:128]
                    nc.tensor.matmul(
                        out=pv[:, :],
                        lhsT=lhsT,
                        rhs=wvS[0:(128 if kc < NK else 1), kc, :],
                        start=(kc == 0), stop=(kc == NK))
                nc.vector.tensor_copy(
                    out=vta[:, m, :, 0:DK],
                    in_=pv[:, :].rearrange("p (h d) -> p h d", h=HL))

            def out_proj(m, eng):
                # phase C: one 128-token output strip
                po2 = psB.tile([128, D], F32, name="po2", tag="big", bufs=2)
                for n2 in range(2):
                    for kc in range(2):
                        nc.tensor.matmul(
                            out=po2[:, n2 * SC:(n2 + 1) * SC],
                            lhsT=xfin[:, kc, m * 128:(m + 1) * 128],
                            rhs=woS[:, kc, n2 * SC:(n2 + 1) * SC],
                            start=(kc == 0), stop=(kc == 1))
                ost = pb.tile([128, D], BF16, name="ost", tag="ost", bufs=4)
                if eng == "act":
                    nc.scalar.activation(ost, po2, AF.Copy)
                else:
                    nc.vector.tensor_copy(out=ost, in_=po2)
                nc.sync.dma_start(out=out[m * 128:(m + 1) * 128, :],
                                  in_=ost)

            vleft = list(range(NJ))  # v strips not yet projected

            def drain_v(n):
                for _ in range(min(n, len(vleft))):
                    emit_vstrip(vleft.pop(0))

            prev_xps = None
            op_m = 0                 # next out-proj strip
            for half in range(2):
                for h in range(HL):
                    po_, hs_ = (h % 2) * DK, h // 2
                    eTs = []
                    for j in range(NJ):
                        eTs.append(emit_smem(h, half, j, po_, hs_))
                        if half == 0 and h == 0 and j == 7:
                            # half-1 k projection rides behind the first
                            # unit's score stream
                            proj_half("k", xk1, wkS, kT, 1, ("dve", "dve"))
                    if half == 0 and h == 0:
                        proj_half("q", xq1, wqS, qT, 1, ("dve", "dve"))
                        drain_v(8)
                    # two acc tiles (1 PSUM bank each) hold the 8 q-chunk
                    # accumulators; a zero-matmul clears each bank so the
                    # per-chunk accumulation can run with start=False
                    accs = []
                    for a in range(2):
                        acc = psB.tile([128, 4, 128], F32, name="acc",
                                       tag="small", bufs=2)
                        nc.tensor.matmul(
                            out=acc[:, :, :], lhsT=zlhs[0:1, 0:128],
                            rhs=ones[0:1, 0:SC],
                            start=True, stop=True)
                        accs.append(acc)
                    if half == 0 and h == 0:
                        drain_v(8)
                    # swapped-orientation PV, strip-incremental
                    for j in range(NJ):
                        for qc in range(NQC):
                            nc.tensor.matmul(
                                out=accs[qc // 4][:, qc % 4, 0:DK + 1],
                                lhsT=eTs[j][:, qc * 128:(qc + 1) * 128],
                                rhs=vta[:, j, h, :],
                                start=False, stop=(j == NJ - 1),
                                skip_group_check=True)
                    # normalize into per-pair token-major x tiles
                    if h % 2 == 0:
                        xps = [pb.tile([128, 2, DK], F16, name="xp",
                                       tag="xp", bufs=16)
                               for _ in range(NQC)]
                        prev_xps = xps
                    else:
                        xps = prev_xps
                    off = half * HS
                    for qc in range(NQC):
                        acc = accs[qc // 4]
                        rec = pb.tile([128, 1], F32, name="rec", tag="rec",
                                      bufs=8)
                        with nc.allow_low_precision(
                                reason="softmax denom reciprocal in fp16"):
                            nc.vector.reciprocal(
                                rec, acc[:, qc % 4, DK:DK + 1])
                            nc.vector.tensor_scalar_mul(
                                xps[qc][:, h % 2, :],
                                acc[:, qc % 4, 0:DK], rec)
                    if h % 2 == 1:
                        # transpose head-pair x tiles to feature-major
                        pair = h // 2
                        for qc in range(NQC):
                            tp = psB.tile([128, 8, 128], F16, name="tp",
                                          tag="small", bufs=2)
                            nc.tensor.transpose(
                                tp[:, 0, :], xps[qc][:, :, :], identS)
                            nc.vector.tensor_copy(
                                out=xfin[:, pair,
                                         off + qc * 128:off + (qc + 1) * 128],
                                in_=tp[:, 0, :])
                    # out-proj strips interleave once their q-half is done
                    if half == 1:
                        while op_m < 2 * (h + 1):
                            out_proj(op_m, "dve")
                            op_m += 1
            for m in range(op_m, NJ):
                out_proj(m, "act" if m % 2 == 0 else "dve")
            pb.release()
            psB.release()
            psA.release()
            pa.release()
            pa2.release()
    nc.finalize()
    return nc


def _get_nc():
    if "nc" not in _CACHE:
        _CACHE["nc"] = _build()
    return _CACHE["nc"]


def _prep_in_maps(query, key_, value, mask, Wq, bq, Wk, bk, Wv, bv, Wo, bo):
    query = np.asarray(query, np.float32)
    key_ = np.asarray(key_, np.float32)
    value = np.asarray(value, np.float32)
    mask = np.asarray(mask)
    Wq, bq = np.asarray(Wq, np.float32), np.asarray(bq, np.float32)
    Wk, bk = np.asarray(Wk, np.float32), np.asarray(bk, np.float32)
    Wv, bv = np.asarray(Wv, np.float32), np.asarray(bv, np.float32)
    Wo = np.asarray(Wo, np.float32)

    xT = {}
    for b in range(B):
        xT[("q", b)] = np.ascontiguousarray(query[b].T).astype(np.float16)
        xT[("k", b)] = np.ascontiguousarray(key_[b].T).astype(np.float16)
        xT[("v", b)] = np.ascontiguousarray(value[b].T).astype(np.float16)
        xT[("m", b)] = np.ascontiguousarray(mask[b].T).astype(
            ml_dtypes.bfloat16)
    identity = np.eye(128, dtype=np.float16)
    wg = {}
    for g in range(GROUPS):
        c0, c1 = g * DH, (g + 1) * DH
        wg[("q", g)] = np.ascontiguousarray(Wq[:, c0:c1]).astype(np.float16)
        wg[("k", g)] = np.ascontiguousarray(Wk[:, c0:c1]).astype(np.float16)
        wg[("v", g)] = np.concatenate(
            [Wv[:, c0:c1], bv[None, c0:c1]], axis=0).astype(np.float16)
        wg[("o", g)] = np.ascontiguousarray(Wo[c0:c1, :]).astype(np.float16)
        wg[("bqk", g)] = np.stack(
            [bq[c0:c0 + 128], bq[c0 + 128:c1],
             bk[c0:c0 + 128], bk[c0 + 128:c1]], axis=1).astype(np.float32)

    in_maps = []
    for c in range(NCORES):
        b, g = c // GROUPS, c % GROUPS
        in_maps.append({
            "xqT": xT[("q", b)], "xkT": xT[("k", b)], "xvT": xT[("v", b)],
            "mT": xT[("m", b)],
            "wq": wg[("q", g)], "wk": wg[("k", g)], "wv": wg[("v", g)],
            "wo": wg[("o", g)], "bqk": wg[("bqk", g)],
            "ident": identity,
        })
    return in_maps


def _gather(results, bo):
    bo = np.asarray(bo, np.float32)
    outs = []
    for b in range(B):
        acc = results[b * GROUPS]["out"].astype(np.float32).copy()
        for g in range(1, GROUPS):
            acc += results[b * GROUPS + g]["out"]
        outs.append(acc + bo[None, :])
    return np.stack(outs, axis=0)


def run(trace=False, **inputs):
    in_maps = _prep_in_maps(**inputs)
    nc = _get_nc()
    res = run_bass_kernel_spmd(nc, in_maps, core_ids=list(range(NCORES)),
                               trace=trace)
    out = _gather(res.results, inputs["bo"])
    return out, res


def kernel(**inputs) -> np.ndarray:
    out, _ = run(trace=False, **inputs)
    return out


# revision 6
# speedup vs baseline: 1.0712x; 1.0712x over previous
"""Multi-head attention on 8 Trainium2 NeuronCores.

Sharding: 2-way data parallel over batch x 4-way tensor parallel over heads
(4 heads/core). Per-core device kernel, for its (batch, head-group):

  phase A : q^T = (x_q @ Wq + bq)^T, k^T likewise (feature-major, fp16),
            pipelined by token half so phase B starts after only the
            q-half0/k-half0 projections; the half-1 projections and the
            v projection ride inside the first two units' score streams.
  phase B : per (q-half, head) unit: s^T = k q^T (transposed-score layout),
            e^T = exp(s^T/8) (bf16), masked in place (DVE/Pool split), then
            PV in the swapped orientation: acc[q, 0:65] += em^T.T @ [v|1]
            per 128-token q-chunk -- the moving operand is the 65-wide
            [v|1], so PV costs 65 PE columns per (strip, chunk) instead of
            1024 per strip, and the ones column gives softmax row-sums for
            free, aligned per-partition.  Normalize via DVE reciprocal +
            per-partition tensor_scalar mul, then PE-transpose head-pair
            x tiles back to feature-major for phase C.  Unit u+1's scores
            are emitted before unit u's PV so the in-order PE queue never
            starves the Activation engine (the exp stream is the global
            bottleneck).
  phase C : partial_out = x^T.T @ Wo_rows (row-parallel Wo), interleaved.

DMAs are batched (multi-strip access patterns) to stay off the SP
sequencer's 565ns-per-DMA serial cost. Host: shards/transposes inputs
(fp16), sums the 4 group partials per batch, adds bo.
"""
import numpy as np
import ml_dtypes

import concourse.bass as bass
import concourse.bacc as bacc
import concourse.tile as tile
from concourse import mybir
from concourse.bass_utils import run_bass_kernel_spmd

B, S, D, H = 2, 2048, 1024, 16
DK = 64                    # head dim
GROUPS = 4                 # head-group tensor parallel factor
HL = H // GROUPS           # heads per core
DH = HL * DK               # 256 local features
NCORES = 8
NK = D // 128              # 8 contraction chunks
NJ = S // 128              # 16 token strips
SC = 512                   # matmul moving-operand chunk
HS = S // 2                # 1024: q-half size in phase B
NQC = HS // 128            # 8 q-chunks per half
F32 = mybir.dt.float32
F16 = mybir.dt.float16
BF16 = mybir.dt.bfloat16
AF = mybir.ActivationFunctionType

# mask-mul strips offloaded from DVE to the (otherwise idle) GPSIMD engine
POOL_JS = (2, 5, 8, 11, 14)

_CACHE = {}


def _build():
    nc = bacc.Bacc("TRN2")
    xqT = nc.dram_tensor("xqT", (D, S), F16, kind="ExternalInput")
    xkT = nc.dram_tensor("xkT", (D, S), F16, kind="ExternalInput")
    xvT = nc.dram_tensor("xvT", (D, S), F16, kind="ExternalInput")
    mT = nc.dram_tensor("mT", (S, S), BF16, kind="ExternalInput")
    wq = nc.dram_tensor("wq", (D, DH), F16, kind="ExternalInput")
    wk = nc.dram_tensor("wk", (D, DH), F16, kind="ExternalInput")
    wv = nc.dram_tensor("wv", (D + 1, DH), F16, kind="ExternalInput")
    wo = nc.dram_tensor("wo", (DH, D), F16, kind="ExternalInput")
    bqk = nc.dram_tensor("bqk", (128, 4), F32, kind="ExternalInput")
    ident = nc.dram_tensor("ident", (128, 128), F16, kind="ExternalInput")
    out = nc.dram_tensor("out", (S, D), BF16, kind="ExternalOutput")

    with tile.TileContext(nc) as tc:
        with tc.tile_pool(name="sp", bufs=1) as sp:
            qT = sp.tile([128, 2, S], F16)
            kT = sp.tile([128, 2, S], F16)
            vta = sp.tile([128, NJ, HL, DK + 1], BF16, name="vta")
            mTs = sp.tile([128, NJ, S], BF16)
            woS = sp.tile([128, 2, D], F16)
            xfin = sp.tile([128, 2, S], F16)
            identS = sp.tile([128, 128], F16)
            ones = sp.tile([1, SC], F16)
            zlhs = sp.tile([1, 128], F16)
            biasT = sp.tile([128, 4], F32)
            nc.vector.memset(ones, 1.0)
            nc.vector.memset(zlhs, 0.0)
            nc.vector.memset(vta[:, :, :, DK:DK + 1], 1.0)

            pa = tc.alloc_tile_pool(name="pa", bufs=1)
            wvS = pa.tile([128, NK + 1, DH], F16, name="wv", bufs=1)
            wqS = pa.tile([128, NK, DH], F16, name="wqS", bufs=1)
            wkS = pa.tile([128, NK, DH], F16, name="wkS", bufs=1)

            def load_w(dst, src):
                nc.sync.dma_start(
                    out=dst[0:128, 0:NK, :],
                    in_=src[0:D, :].rearrange("(kc p) d -> p kc d", p=128))

            def load_xhalf(xT, half):
                """One token-half of an input, batched into one DMA."""
                xs = pa.tile([128, NK, HS], F16, name="xs", tag="xs", bufs=3)
                off = half * HS
                nc.sync.dma_start(
                    out=xs[:, :, :],
                    in_=xT[0:D, off:off + HS].rearrange(
                        "(kc p) s -> p kc s", p=128))
                return xs

            def load_mask(j0, j1, half):
                off = half * HS
                nc.sync.dma_start(
                    out=mTs[:, j0:j1, off:off + HS],
                    in_=mT[j0 * 128:j1 * 128, off:off + HS].rearrange(
                        "(j p) s -> p j s", p=128))

            # --- startup DMA, need-ordered ---
            nc.sync.dma_start(out=biasT, in_=bqk[:, :])
            load_w(wqS, wq)
            xq0 = load_xhalf(xqT, 0)
            load_w(wkS, wk)
            xk0 = load_xhalf(xkT, 0)
            load_mask(0, 4, 0)

            psA = tc.alloc_tile_pool(name="psA", bufs=1, space="PSUM")

            def proj_half(name, xs, wS, dst, half, engs):
                """Project one token-half of q or k: psum [128, HS] per
                m-chunk (serial, projps bufs=1), evict with bias."""
                off = half * HS
                bc0 = 0 if name == "q" else 2
                for m in range(2):
                    ps = psA.tile([128, HS], F32, name=f"ps{name}{half}{m}",
                                  tag="projps", bufs=1)
                    for kc in range(NK):
                        for n in range(HS // SC):
                            nc.tensor.matmul(
                                out=ps[:, n * SC:(n + 1) * SC],
                                lhsT=wS[0:128, kc, m * 128:(m + 1) * 128],
                                rhs=xs[0:128, kc, n * SC:(n + 1) * SC],
                                start=(kc == 0), stop=(kc == NK - 1))
                    if engs[m] == "act":
                        nc.scalar.activation(
                            dst[:, m, off:off + HS], ps, AF.Identity,
                            bias=biasT[:, bc0 + m:bc0 + m + 1])
                    else:
                        with nc.allow_low_precision(
                                reason="bias add into fp16 eviction"):
                            nc.vector.tensor_scalar_add(
                                dst[:, m, off:off + HS], ps,
                                biasT[:, bc0 + m:bc0 + m + 1])

            proj_half("q", xq0, wqS, qT, 0, ("act", "dve"))
            proj_half("k", xk0, wkS, kT, 0, ("act", "dve"))

            # rest of the input stream (SP queue order = need order)
            xk1 = load_xhalf(xkT, 1)
            load_mask(4, 10, 0)
            load_mask(10, NJ, 0)
            nc.sync.dma_start(
                out=wvS[0:128, 0:NK, :],
                in_=wv[0:D, :].rearrange("(kc p) d -> p kc d", p=128))
            nc.sync.dma_start(out=wvS[0:1, NK, :], in_=wv[D:D + 1, :])
            xv0 = load_xhalf(xvT, 0)
            xq1 = load_xhalf(xqT, 1)
            xv1 = load_xhalf(xvT, 1)
            nc.sync.dma_start(
                out=woS[:, :, :],
                in_=wo[:, :].rearrange("(s p) d -> p s d", p=128))
            nc.sync.dma_start(out=identS, in_=ident[:, :])
            load_mask(0, 8, 1)
            load_mask(8, NJ, 1)
            xvs = (xv0, xv1)

            # ---------------- phase B ----------------
            psB = tc.alloc_tile_pool(name="psB", bufs=1, space="PSUM")
            pb = tc.alloc_tile_pool(name="pb", bufs=1)

            units = [(half, h) for half in range(2) for h in range(HL)]
            eTs_of = {}

            def emit_smem(h, half, j, po_, hs_):
                # scores -> exp -> in-place mask-mul for one (unit, j) strip
                off = half * HS
                sT = psB.tile([128, HS], F32, name="sT", tag="big", bufs=2)
                for c2 in range(2):
                    nc.tensor.matmul(
                        out=sT[:, c2 * SC:(c2 + 1) * SC],
                        lhsT=kT[po_:po_ + DK, hs_,
                                j * 128:(j + 1) * 128],
                        rhs=qT[po_:po_ + DK, hs_,
                               off + c2 * SC:off + (c2 + 1) * SC],
                        start=True, stop=True)
                eT = pb.tile([128, HS], BF16, name="eT", tag="eT", bufs=16)
                nc.scalar.activation(eT, sT, AF.Exp, scale=0.125)
                eng = nc.gpsimd if j in POOL_JS else nc.vector
                eng.tensor_mul(eT, eT, mTs[:, j, off:off + HS])
                return eT

            def emit_vstrip(m):
                # v projection for one 128-token strip (token-major + ones);
                # psum rides the otherwise-idle projps bank
                grp, m8 = m // 8, m % 8
                pv = psA.tile([128, DH], F32, name="pv", tag="projps",
                              bufs=1)
                for kc in range(NK + 1):
                    if kc < NK:
                        lhsT = xvs[grp][:, kc, m8 * 128:(m8 + 1) * 128]
                    else:
                        lhsT = ones[0:1, 0:128]
                    nc.tensor.matmul(
                        out=pv[:, :],
                        lhsT=lhsT,
                        rhs=wvS[0:(128 if kc < NK else 1), kc, :],
                        start=(kc == 0), stop=(kc == NK))
                nc.vector.tensor_copy(
                    out=vta[:, m, :, 0:DK],
                    in_=pv[:, :].rearrange("p (h d) -> p h d", h=HL))

            def emit_scores_unit(i):
                half, h = units[i]
                po_, hs_ = (h % 2) * DK, h // 2
                eTs = []
                for j in range(NJ):
                    eTs.append(emit_smem(h, half, j, po_, hs_))
                    if i == 0 and j == 7:
                        # half-1 k projection rides behind unit 0's scores
                        proj_half("k", xk1, wkS, kT, 1, ("dve", "dve"))
                    if i == 1 and 4 <= j < 12:
                        # v projection rides behind unit 1's scores
                        emit_vstrip(2 * (j - 4))
                        emit_vstrip(2 * (j - 4) + 1)
                eTs_of[i] = eTs

            def make_accs():
                # two acc tiles (1 PSUM bank each) hold the 8 q-chunk
                # accumulators; a zero-matmul clears each bank so the
                # per-chunk accumulation can run with start=False
                accs = []
                for a in range(2):
                    acc = psB.tile([128, 4, 128], F32, name="acc",
                                   tag="small", bufs=2)
                    nc.tensor.matmul(
                        out=acc[:, :, :], lhsT=zlhs[0:1, 0:128],
                        rhs=ones[0:1, 0:SC], start=True, stop=True)
                    accs.append(acc)
                return accs

            def emit_pv_strips(i, accs, jr):
                half, h = units[i]
                for j in jr:
                    for qc in range(NQC):
                        nc.tensor.matmul(
                            out=accs[qc // 4][:, qc % 4, 0:DK + 1],
                            lhsT=eTs_of[i][j][:, qc * 128:(qc + 1) * 128],
                            rhs=vta[:, j, h, :],
                            start=False, stop=(j == NJ - 1),
                            skip_group_check=True)

            xps_of = {}

            def emit_norm_unit(i, accs):
                half, h = units[i]
                if h % 2 == 0:
                    xps = [pb.tile([128, 2, DK], F16, name="xp",
                                   tag="xp", bufs=16) for _ in range(NQC)]
                    xps_of[h // 2] = xps
                else:
                    xps = xps_of[h // 2]
                for qc in range(NQC):
                    acc = accs[qc // 4]
                    rec = pb.tile([128, 1], F32, name="rec", tag="rec",
                                  bufs=8)
                    with nc.allow_low_precision(
                            reason="softmax denom reciprocal"):
                        nc.vector.reciprocal(rec, acc[:, qc % 4, DK:DK + 1])
                        nc.vector.tensor_scalar_mul(
                            xps[qc][:, h % 2, :], acc[:, qc % 4, 0:DK], rec)
                if h % 2 == 1:
                    # transpose head-pair x tiles to feature-major xfin
                    pair, off = h // 2, half * HS
                    for qc in range(NQC):
                        tp = psB.tile([128, 8, 128], F16, name="tp",
                                      tag="small", bufs=2)
                        nc.tensor.transpose(
                            tp[:, 0, :], xps[qc][:, :, :], identS)
                        nc.vector.tensor_copy(
                            out=xfin[:, pair,
                                     off + qc * 128:off + (qc + 1) * 128],
                            in_=tp[:, 0, :])

            def out_proj(m, eng):
                # phase C: one 128-token output strip
                po2 = psB.tile([128, D], F32, name="po2", tag="big", bufs=2)
                for n2 in range(2):
                    for kc in range(2):
                        nc.tensor.matmul(
                            out=po2[:, n2 * SC:(n2 + 1) * SC],
                            lhsT=xfin[:, kc, m * 128:(m + 1) * 128],
                            rhs=woS[:, kc, n2 * SC:(n2 + 1) * SC],
                            start=(kc == 0), stop=(kc == 1))
                ost = pb.tile([128, D], BF16, name="ost", tag="ost", bufs=3)
                if eng == "act":
                    nc.scalar.activation(ost, po2, AF.Copy)
                else:
                    nc.vector.tensor_copy(out=ost, in_=po2)
                nc.sync.dma_start(out=out[m * 128:(m + 1) * 128, :],
                                  in_=ost)

            # --- software-pipelined phase B schedule ---
            emit_scores_unit(0)
            emit_scores_unit(1)       # includes the 16 v strips
            accs0 = make_accs()
            emit_pv_strips(0, accs0, range(NJ))
            emit_norm_unit(0, accs0)
            prev = (1, make_accs())
            op_m = 0
            for i in range(2, len(units)):
                emit_scores_unit(i)
                pi, paccs = prev
                emit_pv_strips(pi, paccs, range(NJ))
                emit_norm_unit(pi, paccs)
                if pi == 1:
                    # half-1 q projection, needed from unit 4 on
                    proj_half("q", xq1, wqS, qT, 1, ("dve", "dve"))
                if units[pi][0] == 1:
                    # out-proj strips interleave once half 0 is complete
                    for _ in range(2):
                        out_proj(op_m, "dve")
                        op_m += 1
                prev = (i, make_accs())
            li, laccs = prev
            emit_pv_strips(li, laccs, range(NJ))
            emit_norm_unit(li, laccs)
            for m in range(op_m, NJ):
                out_proj(m, "act" if m % 2 == 0 else "dve")
            pb.release()
            psB.release()
            psA.release()
            pa.release()
    nc.finalize()
    return nc


def _get_nc():
    if "nc" not in _CACHE:
        _CACHE["nc"] = _build()
    return _CACHE["nc"]


def _prep_in_maps(query, key_, value, mask, Wq, bq, Wk, bk, Wv, bv, Wo, bo):
    query = np.asarray(query, np.float32)
    key_ = np.asarray(key_, np.float32)
    value = np.asarray(value, np.float32)
    mask = np.asarray(mask)
    Wq, bq = np.asarray(Wq, np.float32), np.asarray(bq, np.float32)
    Wk, bk = np.asarray(Wk, np.float32), np.asarray(bk, np.float32)
    Wv, bv = np.asarray(Wv, np.float32), np.asarray(bv, np.float32)
    Wo = np.asarray(Wo, np.float32)

    xT = {}
    for b in range(B):
        xT[("q", b)] = np.ascontiguousarray(query[b].T).astype(np.float16)
        xT[("k", b)] = np.ascontiguousarray(key_[b].T).astype(np.float16)
        xT[("v", b)] = np.ascontiguousarray(value[b].T).astype(np.float16)
        xT[("m", b)] = np.ascontiguousarray(mask[b].T).astype(
            ml_dtypes.bfloat16)
    identity = np.eye(128, dtype=np.float16)
    wg = {}
    for g in range(GROUPS):
        c0, c1 = g * DH, (g + 1) * DH
        wg[("q", g)] = np.ascontiguousarray(Wq[:, c0:c1]).astype(np.float16)
        wg[("k", g)] = np.ascontiguousarray(Wk[:, c0:c1]).astype(np.float16)
        wg[("v", g)] = np.concatenate(
            [Wv[:, c0:c1], bv[None, c0:c1]], axis=0).astype(np.float16)
        wg[("o", g)] = np.ascontiguousarray(Wo[c0:c1, :]).astype(np.float16)
        wg[("bqk", g)] = np.stack(
            [bq[c0:c0 + 128], bq[c0 + 128:c1],
             bk[c0:c0 + 128], bk[c0 + 128:c1]], axis=1).astype(np.float32)

    in_maps = []
    for c in range(NCORES):
        b, g = c // GROUPS, c % GROUPS
        in_maps.append({
            "xqT": xT[("q", b)], "xkT": xT[("k", b)], "xvT": xT[("v", b)],
            "mT": xT[("m", b)],
            "wq": wg[("q", g)], "wk": wg[("k", g)], "wv": wg[("v", g)],
            "wo": wg[("o", g)], "bqk": wg[("bqk", g)],
            "ident": identity,
        })
    return in_maps


def _gather(results, bo):
    bo = np.asarray(bo, np.float32)
    outs = []
    for b in range(B):
        acc = results[b * GROUPS]["out"].astype(np.float32).copy()
        for g in range(1, GROUPS):
            acc += results[b * GROUPS + g]["out"]
        outs.append(acc + bo[None, :])
    return np.stack(outs, axis=0)


def run(trace=False, **inputs):
    in_maps = _prep_in_maps(**inputs)
    nc = _get_nc()
    res = run_bass_kernel_spmd(nc, in_maps, core_ids=list(range(NCORES)),
                               trace=trace)
    out = _gather(res.results, inputs["bo"])
    return out, res


def kernel(**inputs) -> np.ndarray:
    out, _ = run(trace=False, **inputs)
    return out


# revision 12
# speedup vs baseline: 1.1793x; 1.1009x over previous
"""Multi-head attention on 8 Trainium2 NeuronCores.

Sharding: 2-way data parallel over batch x 4-way tensor parallel over heads
(4 heads/core). Per-core device kernel, for its (batch, head-group):

  phase A : q^T = (x_q @ Wq + bq)^T, k^T likewise (feature-major, fp16),
            pipelined by token half so phase B starts after only the
            q-half0/k-half0 projections; the half-1 projections and the
            v projection ride inside the first two units' score streams.
  phase B : per (q-half, head) unit: s^T = k q^T (transposed-score layout),
            e^T = exp(s^T/8) (bf16), masked in place (DVE/Pool split), then
            PV in the swapped orientation: acc[q, 0:65] += em^T.T @ [v|1]
            per 128-token q-chunk -- the moving operand is the 65-wide
            [v|1], so PV costs 65 PE columns per (strip, chunk) instead of
            1024 per strip, and the ones column gives softmax row-sums for
            free, aligned per-partition.  Normalize via DVE reciprocal +
            per-partition tensor_scalar mul, then PE-transpose head-pair
            x tiles back to feature-major for phase C.  Unit u+1's scores
            are emitted before unit u's PV so the in-order PE queue never
            starves the Activation engine (the exp stream is the global
            bottleneck).
  phase C : partial_out = x^T.T @ Wo_rows (row-parallel Wo), interleaved.

DMAs are batched (multi-strip access patterns) to stay off the SP
sequencer's 565ns-per-DMA serial cost. Host: shards/transposes inputs
(fp16), sums the 4 group partials per batch, adds bo.
"""
import numpy as np
import ml_dtypes

import concourse.bass as bass
import concourse.bacc as bacc
import concourse.tile as tile
from concourse import mybir
from concourse.bass_utils import run_bass_kernel_spmd

B, S, D, H = 2, 2048, 1024, 16
DK = 64                    # head dim
GROUPS = 4                 # head-group tensor parallel factor
HL = H // GROUPS           # heads per core
DH = HL * DK               # 256 local features
NCORES = 8
NK = D // 128              # 8 contraction chunks
NJ = S // 128              # 16 token strips
SC = 512                   # matmul moving-operand chunk
HS = S // 2                # 1024: q-half size in phase B
NQC = HS // 128            # 8 q-chunks per half
F32 = mybir.dt.float32
F16 = mybir.dt.float16
BF16 = mybir.dt.bfloat16
AF = mybir.ActivationFunctionType

# mask-mul strips offloaded from DVE to the (otherwise idle) GPSIMD engine
POOL_JS = (2, 5, 8, 11, 14)

_CACHE = {}


def _build():
    nc = bacc.Bacc("TRN2")
    xqT = nc.dram_tensor("xqT", (D, S), F16, kind="ExternalInput")
    xkT = nc.dram_tensor("xkT", (D, S), F16, kind="ExternalInput")
    xvT = nc.dram_tensor("xvT", (D, S), F16, kind="ExternalInput")
    mT = nc.dram_tensor("mT", (S, S), BF16, kind="ExternalInput")
    wq = nc.dram_tensor("wq", (D, DH), F16, kind="ExternalInput")
    wk = nc.dram_tensor("wk", (D, DH), F16, kind="ExternalInput")
    wv = nc.dram_tensor("wv", (D + 1, DH), F16, kind="ExternalInput")
    wo = nc.dram_tensor("wo", (DH, D), F16, kind="ExternalInput")
    bqk = nc.dram_tensor("bqk", (128, 4), F32, kind="ExternalInput")
    ident = nc.dram_tensor("ident", (128, 128), F16, kind="ExternalInput")
    out = nc.dram_tensor("out", (S, D), BF16, kind="ExternalOutput")

    with tile.TileContext(nc) as tc:
        with tc.tile_pool(name="sp", bufs=1) as sp:
            qT = sp.tile([128, 2, S], F16)
            kT = sp.tile([128, 2, S], F16)
            vta = sp.tile([128, NJ, HL, DK + 1], BF16, name="vta")
            mTs = sp.tile([128, NJ, S], BF16)
            woS = sp.tile([128, 2, D], F16)
            xfin = sp.tile([128, 2, S], F16)
            identS = sp.tile([128, 128], F16)
            ones = sp.tile([1, SC], F16)
            zlhs = sp.tile([1, 128], F16)
            biasT = sp.tile([128, 4], F32)
            nc.vector.memset(ones, 1.0)
            nc.vector.memset(zlhs, 0.0)
            nc.vector.memset(vta[:, :, :, DK:DK + 1], 1.0)

            pa = tc.alloc_tile_pool(name="pa", bufs=1)
            wvS = pa.tile([128, NK + 1, DH], F16, name="wv", bufs=1)
            wqS = pa.tile([128, NK, DH], F16, name="wqS", bufs=1)
            wkS = pa.tile([128, NK, DH], F16, name="wkS", bufs=1)

            def load_w(dst, src):
                nc.sync.dma_start(
                    out=dst[0:128, 0:NK, :],
                    in_=src[0:D, :].rearrange("(kc p) d -> p kc d", p=128))

            def load_xhalf(xT, half):
                """One token-half of an input, batched into two DMAs so the
                projection can start on the first four chunks."""
                xs = pa.tile([128, NK, HS], F16, name="xs", tag="xs", bufs=3)
                off = half * HS
                for c in range(2):
                    k0 = c * (NK // 2)
                    nc.sync.dma_start(
                        out=xs[:, k0:k0 + NK // 2, :],
                        in_=xT[k0 * 128:(k0 + NK // 2) * 128,
                               off:off + HS].rearrange(
                            "(kc p) s -> p kc s", p=128))
                return xs

            def load_mask(j0, j1, half):
                off = half * HS
                nc.sync.dma_start(
                    out=mTs[:, j0:j1, off:off + HS],
                    in_=mT[j0 * 128:j1 * 128, off:off + HS].rearrange(
                        "(j p) s -> p j s", p=128))

            # --- startup DMA, need-ordered ---
            nc.sync.dma_start(out=biasT, in_=bqk[:, :])
            load_w(wqS, wq)
            xq0 = load_xhalf(xqT, 0)
            load_w(wkS, wk)
            xk0 = load_xhalf(xkT, 0)
            load_mask(0, 4, 0)

            psA = tc.alloc_tile_pool(name="psA", bufs=1, space="PSUM")

            def proj_half(name, xs, wS, dst, half, engs):
                """Project one token-half of q or k. Half-bank psum tiles
                (projps bufs=2) let chunk n+1 project while n evicts."""
                off = half * HS
                bc0 = 0 if name == "q" else 2
                for m in range(2):
                    for n in range(HS // SC):
                        ps = psA.tile([128, SC], F32,
                                      name=f"ps{name}{half}{m}{n}",
                                      tag="projps", bufs=2)
                        for kc in range(NK):
                            nc.tensor.matmul(
                                out=ps[:, :],
                                lhsT=wS[0:128, kc, m * 128:(m + 1) * 128],
                                rhs=xs[0:128, kc, n * SC:(n + 1) * SC],
                                start=(kc == 0), stop=(kc == NK - 1))
                        dslice = dst[:, m, off + n * SC:off + (n + 1) * SC]
                        if engs[n % 2] == "act":
                            nc.scalar.activation(
                                dslice, ps, AF.Identity,
                                bias=biasT[:, bc0 + m:bc0 + m + 1])
                        else:
                            with nc.allow_low_precision(
                                    reason="bias add into fp16 eviction"):
                                nc.vector.tensor_scalar_add(
                                    dslice, ps,
                                    biasT[:, bc0 + m:bc0 + m + 1])

            proj_half("q", xq0, wqS, qT, 0, ("act", "dve"))
            proj_half("k", xk0, wkS, kT, 0, ("act", "dve"))

            # rest of the input stream (SP queue order = need order)
            xk1 = load_xhalf(xkT, 1)
            load_mask(4, 10, 0)
            load_mask(10, NJ, 0)
            nc.sync.dma_start(
                out=wvS[0:128, 0:NK, :],
                in_=wv[0:D, :].rearrange("(kc p) d -> p kc d", p=128))
            nc.sync.dma_start(out=wvS[0:1, NK, :], in_=wv[D:D + 1, :])
            xv0 = load_xhalf(xvT, 0)
            xq1 = load_xhalf(xqT, 1)
            xv1 = load_xhalf(xvT, 1)
            nc.sync.dma_start(
                out=woS[:, :, :],
                in_=wo[:, :].rearrange("(s p) d -> p s d", p=128))
            nc.sync.dma_start(out=identS, in_=ident[:, :])
            load_mask(0, 8, 1)
            load_mask(8, NJ, 1)
            xvs = (xv0, xv1)

            # ---------------- phase B ----------------
            psB = tc.alloc_tile_pool(name="psB", bufs=1, space="PSUM")
            pb = tc.alloc_tile_pool(name="pb", bufs=1)

            units = [(half, h) for half in range(2) for h in range(HL)]
            eTs_of = {}

            def emit_smem(h, half, j, po_, hs_):
                # scores -> exp -> in-place mask-mul for one (unit, j) strip
                off = half * HS
                sT = psB.tile([128, HS], F32, name="sT", tag="big", bufs=2)
                for c2 in range(2):
                    nc.tensor.matmul(
                        out=sT[:, c2 * SC:(c2 + 1) * SC],
                        lhsT=kT[po_:po_ + DK, hs_,
                                j * 128:(j + 1) * 128],
                        rhs=qT[po_:po_ + DK, hs_,
                               off + c2 * SC:off + (c2 + 1) * SC],
                        start=True, stop=True)
                eT = pb.tile([128, HS], BF16, name="eT", tag="eT", bufs=16)
                nc.scalar.activation(eT, sT, AF.Exp, scale=0.125)
                eng = nc.gpsimd if j in POOL_JS else nc.vector
                eng.tensor_mul(eT, eT, mTs[:, j, off:off + HS])
                return eT

            def emit_vstrip(m):
                # v projection for one 128-token strip (token-major + ones);
                # psum rides the otherwise-idle projps bank
                grp, m8 = m // 8, m % 8
                pv = psA.tile([128, DH], F32, name="pv", tag="projps",
                              bufs=2)
                for kc in range(NK + 1):
                    if kc < NK:
                        lhsT = xvs[grp][:, kc, m8 * 128:(m8 + 1) * 128]
                    else:
                        lhsT = ones[0:1, 0:128]
                    nc.tensor.matmul(
                        out=pv[:, :],
                        lhsT=lhsT,
                        rhs=wvS[0:(128 if kc < NK else 1), kc, :],
                        start=(kc == 0), stop=(kc == NK))
                nc.vector.tensor_copy(
                    out=vta[:, m, :, 0:DK],
                    in_=pv[:, :].rearrange("p (h d) -> p h d", h=HL))

            def emit_scores_unit(i):
                half, h = units[i]
                po_, hs_ = (h % 2) * DK, h // 2
                eTs = []
                for j in range(NJ):
                    eTs.append(emit_smem(h, half, j, po_, hs_))
                    if i == 0 and j == 7:
                        # half-1 k projection rides behind unit 0's scores
                        proj_half("k", xk1, wkS, kT, 1, ("dve", "dve"))
                    if i == 1 and 4 <= j < 12:
                        # v projection rides behind unit 1's scores
                        emit_vstrip(2 * (j - 4))
                        emit_vstrip(2 * (j - 4) + 1)
                eTs_of[i] = eTs

            def make_accs():
                # two acc tiles (1 PSUM bank each) hold the 8 q-chunk
                # accumulators; a zero-matmul clears each bank so the
                # per-chunk accumulation can run with start=False
                accs = []
                for a in range(2):
                    acc = psB.tile([128, 4, 128], F32, name="acc",
                                   tag="small", bufs=2)
                    nc.tensor.matmul(
                        out=acc[:, :, :], lhsT=zlhs[0:1, 0:128],
                        rhs=ones[0:1, 0:SC], start=True, stop=True)
                    accs.append(acc)
                return accs

            def emit_pv_strips(i, accs, jr):
                half, h = units[i]
                for j in jr:
                    for qc in range(NQC):
                        nc.tensor.matmul(
                            out=accs[qc // 4][:, qc % 4, 0:DK + 1],
                            lhsT=eTs_of[i][j][:, qc * 128:(qc + 1) * 128],
                            rhs=vta[:, j, h, :],
                            start=False, stop=(j == NJ - 1),
                            skip_group_check=True)

            xps_of = {}

            def emit_norm_unit(i, accs):
                half, h = units[i]
                if h % 2 == 0:
                    xps = [pb.tile([128, 2, DK], F16, name="xp",
                                   tag="xp", bufs=16) for _ in range(NQC)]
                    xps_of[h // 2] = xps
                else:
                    xps = xps_of[h // 2]
                for qc in range(NQC):
                    acc = accs[qc // 4]
                    rec = pb.tile([128, 1], F32, name="rec", tag="rec",
                                  bufs=8)
                    with nc.allow_low_precision(
                            reason="softmax denom reciprocal"):
                        nc.vector.reciprocal(rec, acc[:, qc % 4, DK:DK + 1])
                        nc.vector.tensor_scalar_mul(
                            xps[qc][:, h % 2, :], acc[:, qc % 4, 0:DK], rec)
                if h % 2 == 1:
                    # transpose head-pair x tiles to feature-major xfin
                    pair, off = h // 2, half * HS
                    for qc in range(NQC):
                        tp = psB.tile([128, 8, 128], F16, name="tp",
                                      tag="small", bufs=2)
                        nc.tensor.transpose(
                            tp[:, 0, :], xps[qc][:, :, :], identS)
                        nc.vector.tensor_copy(
                            out=xfin[:, pair,
                                     off + qc * 128:off + (qc + 1) * 128],
                            in_=tp[:, 0, :])

            def out_proj(m, engs):
                # phase C: one 128-token output strip; half-bank psum tiles
                # on the "small" ring keep the sT double-buffer undisturbed
                ost = pb.tile([128, D], BF16, name="ost", tag="ost", bufs=3)
                for n2 in range(2):
                    po = psB.tile([128, SC], F32, name="po2",
                                  tag="small", bufs=2)
                    for kc in range(2):
                        nc.tensor.matmul(
                            out=po,
                            lhsT=xfin[:, kc, m * 128:(m + 1) * 128],
                            rhs=woS[:, kc, n2 * SC:(n2 + 1) * SC],
                            start=(kc == 0), stop=(kc == 1))
                    oslice = ost[:, n2 * SC:(n2 + 1) * SC]
                    if engs[n2] == "act":
                        nc.scalar.activation(oslice, po, AF.Copy)
                    else:
                        nc.vector.tensor_copy(out=oslice, in_=po)
                nc.sync.dma_start(out=out[m * 128:(m + 1) * 128, :],
                                  in_=ost)

            # --- software-pipelined phase B schedule ---
            # Unit u's scores are emitted between the two halves of unit
            # u-1's PV so the eT ring slots for u's first exps free up
            # before u's score stream occupies the PE queue.
            emit_scores_unit(0)
            emit_scores_unit(1)       # includes the 16 v strips
            accs0 = make_accs()
            emit_pv_strips(0, accs0, range(NJ))
            emit_norm_unit(0, accs0)
            prev = (1, make_accs())
            op_m = 0
            for i in range(2, len(units)):
                pi, paccs = prev
                emit_pv_strips(pi, paccs, range(0, NJ // 2))
                emit_scores_unit(i)
                emit_pv_strips(pi, paccs, range(NJ // 2, NJ))
                emit_norm_unit(pi, paccs)
                if pi == 1:
                    # half-1 q projection, needed from unit 4 on
                    proj_half("q", xq1, wqS, qT, 1, ("dve", "dve"))
                if units[pi][0] == 1:
                    # out-proj strips interleave once half 0 is complete
                    for _ in range(2):
                        out_proj(op_m, ("dve", "dve"))
                        op_m += 1
                prev = (i, make_accs())
            li, laccs = prev
            emit_pv_strips(li, laccs, range(NJ))
            emit_norm_unit(li, laccs)
            for m in range(op_m, NJ):
                out_proj(m, ("act", "dve"))
            pb.release()
            psB.release()
            psA.release()
            pa.release()
    nc.finalize()
    return nc


def _get_nc():
    if "nc" not in _CACHE:
        _CACHE["nc"] = _build()
    return _CACHE["nc"]


def _prep_in_maps(query, key_, value, mask, Wq, bq, Wk, bk, Wv, bv, Wo, bo):
    query = np.asarray(query, np.float32)
    key_ = np.asarray(key_, np.float32)
    value = np.asarray(value, np.float32)
    mask = np.asarray(mask)
    Wq, bq = np.asarray(Wq, np.float32), np.asarray(bq, np.float32)
    Wk, bk = np.asarray(Wk, np.float32), np.asarray(bk, np.float32)
    Wv, bv = np.asarray(Wv, np.float32), np.asarray(bv, np.float32)
    Wo = np.asarray(Wo, np.float32)

    xT = {}
    for b in range(B):
        xT[("q", b)] = np.ascontiguousarray(query[b].T).astype(np.float16)
        xT[("k", b)] = np.ascontiguousarray(key_[b].T).astype(np.float16)
        xT[("v", b)] = np.ascontiguousarray(value[b].T).astype(np.float16)
        xT[("m", b)] = np.ascontiguousarray(mask[b].T).astype(
            ml_dtypes.bfloat16)
    identity = np.eye(128, dtype=np.float16)
    wg = {}
    for g in range(GROUPS):
        c0, c1 = g * DH, (g + 1) * DH
        wg[("q", g)] = np.ascontiguousarray(Wq[:, c0:c1]).astype(np.float16)
        wg[("k", g)] = np.ascontiguousarray(Wk[:, c0:c1]).astype(np.float16)
        wg[("v", g)] = np.concatenate(
            [Wv[:, c0:c1], bv[None, c0:c1]], axis=0).astype(np.float16)
        wg[("o", g)] = np.ascontiguousarray(Wo[c0:c1, :]).astype(np.float16)
        wg[("bqk", g)] = np.stack(
            [bq[c0:c0 + 128], bq[c0 + 128:c1],
             bk[c0:c0 + 128], bk[c0 + 128:c1]], axis=1).astype(np.float32)

    in_maps = []
    for c in range(NCORES):
        b, g = c // GROUPS, c % GROUPS
        in_maps.append({
            "xqT": xT[("q", b)], "xkT": xT[("k", b)], "xvT": xT[("v", b)],
            "mT": xT[("m", b)],
            "wq": wg[("q", g)], "wk": wg[("k", g)], "wv": wg[("v", g)],
            "wo": wg[("o", g)], "bqk": wg[("bqk", g)],
            "ident": identity,
        })
    return in_maps


def _gather(results, bo):
    bo = np.asarray(bo, np.float32)
    outs = []
    for b in range(B):
        acc = results[b * GROUPS]["out"].astype(np.float32).copy()
        for g in range(1, GROUPS):
            acc += results[b * GROUPS + g]["out"]
        outs.append(acc + bo[None, :])
    return np.stack(outs, axis=0)


def run(trace=False, **inputs):
    in_maps = _prep_in_maps(**inputs)
    nc = _get_nc()
    res = run_bass_kernel_spmd(nc, in_maps, core_ids=list(range(NCORES)),
                               trace=trace)
    out = _gather(res.results, inputs["bo"])
    return out, res


def kernel(**inputs) -> np.ndarray:
    out, _ = run(trace=False, **inputs)
    return out
